# revision 11
# baseline (speedup 1.0000x reference)
"""Multi-head self-attention with RoPE + causal mask on 8 Trainium2 NeuronCores.

Sharding: tensor-parallel over heads. Each of the 8 cores owns 2 of the 16
heads (a 128-wide slice of the QKV output dim / Wo input dim) and processes
all 4 batches. Each core computes a partial output x_out_c = O_c @ Wo_c^T of
the full output shape; the host sums the 8 partials (the Wo row-split
all-reduce done host-side at gather time).

Per-core math (all matmuls in float32r on the PE; fp32 elsewhere):
  - Host passes x pre-transposed (xT) so the D-contraction lands on SBUF
    partitions without any device-side transpose of x.
  - Q^T/K^T are produced in [128 head-dims, S] layout; RoPE is applied with a
    sign-folded sin table + a +-32-partition swap done by a constant
    permutation matmul on the PE; the RoPE de-interleave (even dims first,
    odd dims second, per head) is folded into the rows of Wq/Wk on the host
    (attention scores are invariant to a shared permutation of q/k dims).
  - Scores are computed TRANSPOSED: scoresT[sk, sq] = K^T.T @ Q^T, two heads
    packed into one pass via row-tiled K=64 matmuls. exp runs on ScalarE
    straight out of PSUM (scale=1/8 folded in; inputs are bounded so no max
    subtraction is needed). Causal masking only touches the 128x128
    true-diagonal triangles (0/1 mask multiply).
  - P.V uses exp tiles directly as the moving operand (no transposes);
    V is augmented with a ones column so the softmax denominators fall out
    of the same matmul for free. O^T = V_aug^T @ P^T accumulates in PSUM;
    normalization multiplies by the broadcast reciprocal denominator row.
  - Output projection contracts the 128 head dims: out[sq,:] = O_c^T.T @ Wo_c^T.
"""

import sys

sys.path.insert(0, "/opt/trn_rl_repo")

import numpy as np
from contextlib import ExitStack

import concourse.bass as bass
import concourse.tile as tile
from concourse import bacc, mybir
from concourse.bass_utils import run_bass_kernel_spmd

F32 = mybir.dt.float32
F32R = mybir.dt.float32r

# problem constants (hardcoded per harness contract)
B = 4
S = 2048
D = 1024
NUM_HEADS = 16
DK = 64
THETA = 10000.0
NCORES = 8
HPC = NUM_HEADS // NCORES  # 2 heads per core
BLK = HPC * DK  # 128-wide per-core head-dim block
P = 128
SQT = 512  # sq tile width
NKC = D // P  # 8 contraction chunks for the projections


def build_program(b=B, s=S):
    """Build the (SPMD-shared) per-core Bass program."""
    nc = bacc.Bacc("TRN2", target_bir_lowering=False, debug=False)

    n_sqt = s // SQT  # sq tiles per batch
    n_skc = s // P  # sk chunks per batch
    n_st = s // P  # sq chunks for the output projection

    # ---- DRAM I/O ----
    xT = nc.dram_tensor("xT", [b, NKC, P, s], F32R, kind="ExternalInput").ap()
    wqT = nc.dram_tensor("wqT", [NKC, P, BLK], F32R, kind="ExternalInput").ap()
    wkT = nc.dram_tensor("wkT", [NKC, P, BLK], F32R, kind="ExternalInput").ap()
    wvT = nc.dram_tensor("wvT", [NKC, P, BLK], F32R, kind="ExternalInput").ap()
    woT = nc.dram_tensor("woT", [BLK, D], F32R, kind="ExternalInput").ap()
    costab = nc.dram_tensor("costab", [P, s], F32R, kind="ExternalInput").ap()
    sintab = nc.dram_tensor("sintab", [P, s], F32R, kind="ExternalInput").ap()
    pmswap = nc.dram_tensor("pmswap", [P, P], F32R, kind="ExternalInput").ap()
    causal = nc.dram_tensor("causal", [P, P], F32R, kind="ExternalInput").ap()
    ident = nc.dram_tensor("ident", [P, DK], F32R, kind="ExternalInput").ap()
    onescol = nc.dram_tensor("onescol", [P, 1], F32R, kind="ExternalInput").ap()
    out = nc.dram_tensor("out", [b, s, D], F32, kind="ExternalOutput").ap()

    with tile.TileContext(nc) as tc, ExitStack() as ctx:
        consts = ctx.enter_context(tc.tile_pool(name="consts", bufs=1))
        xpool = ctx.enter_context(tc.tile_pool(name="xpool", bufs=1))
        qkv = ctx.enter_context(tc.tile_pool(name="qkv", bufs=1))
        work = ctx.enter_context(tc.tile_pool(name="work", bufs=2))
        psum = ctx.enter_context(tc.tile_pool(name="psum", bufs=2, space="PSUM"))
        opsum = ctx.enter_context(tc.tile_pool(name="opsum", bufs=1, space="PSUM"))

        # ---- constants resident in SBUF ----
        w_sb = {}
        for name, ap in (("wq", wqT), ("wk", wkT), ("wv", wvT)):
            t = consts.tile([P, NKC, BLK], F32R, tag=f"w_{name}")
            for kc in range(NKC):
                nc.sync.dma_start(t[:, kc], ap[kc])
            w_sb[name] = t
        wo_sb = consts.tile([BLK, D], F32R, tag="wo")
        nc.sync.dma_start(wo_sb[:], woT)
        cos_sb = consts.tile([P, s], F32R, tag="cos")
        nc.sync.dma_start(cos_sb[:], costab)
        sin_sb = consts.tile([P, s], F32R, tag="sin")
        nc.sync.dma_start(sin_sb[:], sintab)
        pm_sb = consts.tile([P, P], F32R, tag="pm")
        nc.sync.dma_start(pm_sb[:], pmswap)
        ca_sb = consts.tile([P, P], F32R, tag="causal")
        nc.sync.dma_start(ca_sb[:], causal)
        id_sb = consts.tile([P, DK], F32R, tag="ident")
        nc.sync.dma_start(id_sb[:], ident)
        ones_sb = consts.tile([P, 1], F32R, tag="ones")
        nc.sync.dma_start(ones_sb[:], onescol)

        for bi in range(b):
            # ---- load xT chunks for this batch ----
            xt = []
            for kc in range(NKC):
                t = xpool.tile([P, s], F32R, tag=f"xt{kc}")
                nc.sync.dma_start(t[:], xT[bi, kc])
                xt.append(t)

            # ---- QKV projections -> [128 dims, s] each ----
            qkv_sb = {}
            for name in ("wq", "wk", "wv"):
                dst = qkv.tile([P, s], F32R, tag=f"{name}_sb")
                for nt in range(s // SQT):
                    ps = psum.tile([P, SQT], F32, tag="mm_ps")
                    for kc in range(NKC):
                        nc.tensor.matmul(
                            ps[:],
                            w_sb[name][:, kc],
                            xt[kc][:, nt * SQT : (nt + 1) * SQT],
                            start=(kc == 0),
                            stop=(kc == NKC - 1),
                        )
                    nc.any.tensor_copy(dst[:, nt * SQT : (nt + 1) * SQT], ps[:])
                qkv_sb[name] = dst

            # ---- RoPE on Q^T and K^T (in place) ----
            for name in ("wq", "wk"):
                t_ = qkv_sb[name]
                for nt in range(s // SQT):
                    w = slice(nt * SQT, (nt + 1) * SQT)
                    ps_sw = psum.tile([P, SQT], F32, tag="mm_ps")
                    nc.tensor.matmul(ps_sw[:], pm_sb[:], t_[:, w], start=True, stop=True)
                    t_sin = work.tile([P, SQT], F32R, tag="t_sin")
                    nc.vector.tensor_tensor(
                        t_sin[:], ps_sw[:], sin_sb[:, w], mybir.AluOpType.mult
                    )
                    t_cos = work.tile([P, SQT], F32R, tag="t_cos")
                    nc.vector.tensor_tensor(
                        t_cos[:], t_[:, w], cos_sb[:, w], mybir.AluOpType.mult
                    )
                    nc.vector.tensor_tensor(
                        t_[:, w], t_cos[:], t_sin[:], mybir.AluOpType.add
                    )

            # ---- V: transpose V^T -> V_aug chunks [128 sk, 65] (ones col) ----
            v_aug = []
            for skc in range(n_skc):
                t = qkv.tile([P, HPC, DK + 1], F32R, tag=f"vaug{skc % 16}")
                for h in range(HPC):
                    ps_t = psum.tile([P, DK], F32R, tag="mm_ps")
                    nc.tensor.transpose(
                        ps_t[:],
                        qkv_sb["wv"][h * DK : (h + 1) * DK, skc * P : (skc + 1) * P],
                        id_sb[h * DK : (h + 1) * DK, 0:DK],
                    )
                    nc.any.tensor_copy(t[:, h, 0:DK], ps_t[:])
                    nc.vector.tensor_copy(t[:, h, DK : DK + 1], ones_sb[:])
                v_aug.append(t)

            # ---- attention per sq tile ----
            oT = qkv.tile([P, s], F32R, tag="oT")
            for sqt in range(n_sqt):
                sq0 = sqt * SQT
                nsk = (sq0 + SQT) // P  # causal: sk chunks touching this tile
                po = [
                    opsum.tile([DK + 1, SQT], F32, tag=f"po{h}", name=f"po{h}")
                    for h in range(HPC)
                ]
                for skc in range(nsk):
                    off = max(0, skc * P - sq0)
                    wlen = SQT - off
                    exp_t = [None, None]
                    for h in range(HPC):
                        hd = slice(h * DK, (h + 1) * DK)
                        ps_s = psum.tile([P, SQT], F32, tag=f"score{h}")
                        nc.tensor.matmul(
                            ps_s[:, off:SQT],
                            qkv_sb["wk"][hd, skc * P : (skc + 1) * P],
                            qkv_sb["wq"][hd, sq0 + off : sq0 + SQT],
                            start=True,
                            stop=True,
                        )
                        et = work.tile([P, SQT], F32R, tag=f"exp{h}")
                        nc.scalar.activation(
                            et[:, off:SQT],
                            ps_s[:, off:SQT],
                            mybir.ActivationFunctionType.Exp,
                            scale=float(1.0 / np.sqrt(DK)),
                        )
                        if skc * P >= sq0:  # diagonal chunk: mask the triangle
                            nc.vector.tensor_tensor(
                                et[:, off : off + P],
                                et[:, off : off + P],
                                ca_sb[:],
                                mybir.AluOpType.mult,
                            )
                        exp_t[h] = et
                    for h in range(HPC):
                        nc.tensor.matmul(
                            po[h][:, off:SQT],
                            v_aug[skc][:, h],
                            exp_t[h][:, off:SQT],
                            start=(skc == 0),
                            stop=(skc == nsk - 1),
                        )
                # normalize: oT[h, :, sq] = po[h][0:64, sq] / po[h][64, sq]
                for h in range(HPC):
                    den = work.tile([1, SQT], F32, tag="den")
                    nc.any.tensor_copy(den[:], po[h][DK : DK + 1, :])
                    rec = work.tile([1, SQT], F32, tag="rec")
                    nc.vector.reciprocal(rec[:], den[:])
                    rec_bc = work.tile([DK, SQT], F32, tag="rec_bc")
                    nc.gpsimd.partition_broadcast(rec_bc[:], rec[:])
                    nc.vector.tensor_tensor(
                        oT[h * DK : (h + 1) * DK, sq0 : sq0 + SQT],
                        po[h][0:DK, :],
                        rec_bc[:],
                        mybir.AluOpType.mult,
                    )

            # ---- output projection: out[sq, :] += oT[:, sq].T @ woT ----
            for st in range(n_st):
                ob = work.tile([P, D], F32, tag="ob")
                for nt in range(D // SQT):
                    ps_p = psum.tile([P, SQT], F32, tag="mm_ps")
                    nc.tensor.matmul(
                        ps_p[:],
                        oT[:, st * P : (st + 1) * P],
                        wo_sb[:, nt * SQT : (nt + 1) * SQT],
                        start=True,
                        stop=True,
                    )
                    nc.any.tensor_copy(ob[:, nt * SQT : (nt + 1) * SQT], ps_p[:])
                nc.sync.dma_start(out[bi, st * P : (st + 1) * P, :], ob[:])

    nc.compile()
    return nc


# ---------------- host side ----------------

_ROPE_PERM = None


def _rope_perm():
    """Per-head de-interleave: even dims first, then odd dims."""
    global _ROPE_PERM
    if _ROPE_PERM is None:
        p = []
        for h in range(HPC):
            base = h * DK
            p += [base + 2 * k for k in range(DK // 2)]
            p += [base + 2 * k + 1 for k in range(DK // 2)]
        _ROPE_PERM = np.array(p)
    return _ROPE_PERM


def _host_tables(token_positions, s):
    pos = np.asarray(token_positions).astype(np.float64)
    freqs = THETA ** (-np.arange(0, DK, 2, dtype=np.float64) / DK)  # [32]
    ang = pos[None, :] * freqs[:, None]  # [32, s]
    cos32 = np.cos(ang)
    sin32 = np.sin(ang)
    # layout [128, s]: per head block of 64: [cos32 (x1 half); cos32 (x2 half)]
    cos_t = np.empty((P, s), np.float32)
    sin_t = np.empty((P, s), np.float32)
    for h in range(HPC):
        b0 = h * DK
        cos_t[b0 : b0 + 32] = cos32
        cos_t[b0 + 32 : b0 + 64] = cos32
        sin_t[b0 : b0 + 32] = -sin32  # x1 half: -sin * x2
        sin_t[b0 + 32 : b0 + 64] = sin32  # x2 half: +sin * x1
    # swap permutation matrix (symmetric): swap(j) = j+-32 within each 64-block
    pm = np.zeros((P, P), np.float32)
    for h in range(HPC):
        b0 = h * DK
        for k in range(32):
            pm[b0 + k + 32, b0 + k] = 1.0
            pm[b0 + k, b0 + k + 32] = 1.0
    return cos_t, sin_t, pm


_NC_CACHE = {}

# test harness hooks (off by default; harness calls kernel() directly)
TRACE = False
LAST = {}


def _get_program(b, s):
    key = (b, s)
    if key not in _NC_CACHE:
        _NC_CACHE[key] = build_program(b, s)
    return _NC_CACHE[key]


def prepare_in_maps(x, Wq, Wk, Wv, Wo, token_positions):
    x = np.asarray(x, dtype=np.float32)
    Wq = np.asarray(Wq, dtype=np.float32)
    Wk = np.asarray(Wk, dtype=np.float32)
    Wv = np.asarray(Wv, dtype=np.float32)
    Wo = np.asarray(Wo, dtype=np.float32)
    b, s, _ = x.shape

    # [b, kc, p, s] transposed view of x
    xT = np.ascontiguousarray(
        x.transpose(0, 2, 1).reshape(b, NKC, P, s), dtype=np.float32
    )
    cos_t, sin_t, pm = _host_tables(token_positions, s)
    causal = np.triu(np.ones((P, P), np.float32))  # keep p <= f
    ident = np.tile(np.eye(DK, dtype=np.float32), (HPC, 1))

    perm = _rope_perm()
    in_maps = []
    for c in range(NCORES):
        rows = slice(c * BLK, (c + 1) * BLK)
        wq_c = Wq[rows][perm]  # [128, D] rope-permuted rows
        wk_c = Wk[rows][perm]
        wv_c = Wv[rows]
        in_maps.append(
            {
                "xT": xT,
                "wqT": np.ascontiguousarray(wq_c.T.reshape(NKC, P, BLK)),
                "wkT": np.ascontiguousarray(wk_c.T.reshape(NKC, P, BLK)),
                "wvT": np.ascontiguousarray(wv_c.T.reshape(NKC, P, BLK)),
                "woT": np.ascontiguousarray(Wo[:, rows].T),
                "costab": cos_t,
                "sintab": sin_t,
                "pmswap": pm,
                "causal": causal,
                "ident": ident,
                "onescol": np.ones((P, 1), np.float32),
            }
        )

    return in_maps


def kernel(x, Wq, Wk, Wv, Wo, token_positions):
    b, s, _ = np.asarray(x).shape
    nc = _get_program(b, s)
    in_maps = prepare_in_maps(x, Wq, Wk, Wv, Wo, token_positions)
    res = run_bass_kernel_spmd(
        nc, in_maps, core_ids=list(range(NCORES)), trace=TRACE
    )
    LAST["exec_time_ns"] = res.exec_time_ns
    LAST["profile_json"] = res.profile_json
    acc = res.results[0]["out"].astype(np.float32)
    for c in range(1, NCORES):
        acc += res.results[c]["out"]
    return acc


# revision 12
# speedup vs baseline: 95.6289x; 95.6289x over previous
"""Multi-head self-attention with RoPE + causal mask on 8 Trainium2 NeuronCores.

Sharding: tensor-parallel over heads. Each of the 8 cores owns 2 of the 16
heads (a 128-wide slice of the QKV output dim / Wo input dim) and processes
all 4 batches. Each core computes a partial output x_out_c = O_c @ Wo_c^T of
the full output shape; the host sums the 8 partials (the Wo row-split
all-reduce done host-side at gather time).

Per-core math (all matmuls in float32r on the PE; fp32 elsewhere):
  - Host passes x pre-transposed (xT) so the D-contraction lands on SBUF
    partitions without any device-side transpose of x.
  - Q^T/K^T are produced in [128 head-dims, S] layout; RoPE is applied with a
    sign-folded sin table + a +-32-partition swap done by a constant
    permutation matmul on the PE; the RoPE de-interleave (even dims first,
    odd dims second, per head) is folded into the rows of Wq/Wk on the host
    (attention scores are invariant to a shared permutation of q/k dims).
  - Scores are computed TRANSPOSED: scoresT[sk, sq] = K^T.T @ Q^T, two heads
    packed into one pass via row-tiled K=64 matmuls. exp runs on ScalarE
    straight out of PSUM (scale=1/8 folded in; inputs are bounded so no max
    subtraction is needed). Causal masking only touches the 128x128
    true-diagonal triangles (0/1 mask multiply).
  - P.V uses exp tiles directly as the moving operand (no transposes);
    V is augmented with a ones column so the softmax denominators fall out
    of the same matmul for free. O^T = V_aug^T @ P^T accumulates in PSUM;
    normalization multiplies by the broadcast reciprocal denominator row.
  - Output projection contracts the 128 head dims: out[sq,:] = O_c^T.T @ Wo_c^T.
"""

import sys

sys.path.insert(0, "/opt/trn_rl_repo")

import numpy as np
from contextlib import ExitStack

import concourse.bass as bass
import concourse.tile as tile
from concourse import bacc, mybir
from concourse.bass_utils import run_bass_kernel_spmd

F32 = mybir.dt.float32
F32R = mybir.dt.float32r

# problem constants (hardcoded per harness contract)
B = 4
S = 2048
D = 1024
NUM_HEADS = 16
DK = 64
THETA = 10000.0
NCORES = 8
HPC = NUM_HEADS // NCORES  # 2 heads per core
BLK = HPC * DK  # 128-wide per-core head-dim block
P = 128
SQT = 512  # sq tile width
NKC = D // P  # 8 contraction chunks for the projections


def build_program(b=B, s=S, reps=1):
    """Build the (SPMD-shared) per-core Bass program.

    reps>1 repeats the whole computation (for marginal-cost timing)."""
    nc = bacc.Bacc("TRN2", target_bir_lowering=False, debug=False)

    n_sqt = s // SQT  # sq tiles per batch
    n_skc = s // P  # sk chunks per batch
    n_st = s // P  # sq chunks for the output projection

    # ---- DRAM I/O ----
    xT = nc.dram_tensor("xT", [b, NKC, P, s], F32R, kind="ExternalInput").ap()
    wqT = nc.dram_tensor("wqT", [NKC, P, BLK], F32R, kind="ExternalInput").ap()
    wkT = nc.dram_tensor("wkT", [NKC, P, BLK], F32R, kind="ExternalInput").ap()
    wvT = nc.dram_tensor("wvT", [NKC, P, BLK], F32R, kind="ExternalInput").ap()
    woT = nc.dram_tensor("woT", [BLK, D], F32R, kind="ExternalInput").ap()
    costab = nc.dram_tensor("costab", [P, s], F32R, kind="ExternalInput").ap()
    sintab = nc.dram_tensor("sintab", [P, s], F32R, kind="ExternalInput").ap()
    pmswap = nc.dram_tensor("pmswap", [P, P], F32R, kind="ExternalInput").ap()
    causal = nc.dram_tensor("causal", [P, P], F32R, kind="ExternalInput").ap()
    ident = nc.dram_tensor("ident", [P, DK], F32R, kind="ExternalInput").ap()
    onescol = nc.dram_tensor("onescol", [P, 1], F32R, kind="ExternalInput").ap()
    out = nc.dram_tensor("out", [b, s, D], F32, kind="ExternalOutput").ap()

    with tile.TileContext(nc) as tc, ExitStack() as ctx:
        consts = ctx.enter_context(tc.tile_pool(name="consts", bufs=1))
        xpool = ctx.enter_context(tc.tile_pool(name="xpool", bufs=1))
        qkv = ctx.enter_context(tc.tile_pool(name="qkv", bufs=1))
        work = ctx.enter_context(tc.tile_pool(name="work", bufs=2))
        psum = ctx.enter_context(tc.tile_pool(name="psum", bufs=2, space="PSUM"))
        opsum = ctx.enter_context(tc.tile_pool(name="opsum", bufs=1, space="PSUM"))

        # ---- constants resident in SBUF ----
        w_sb = {}
        for name, ap in (("wq", wqT), ("wk", wkT), ("wv", wvT)):
            t = consts.tile([P, NKC, BLK], F32R, tag=f"w_{name}")
            for kc in range(NKC):
                nc.sync.dma_start(t[:, kc], ap[kc])
            w_sb[name] = t
        wo_sb = consts.tile([BLK, D], F32R, tag="wo")
        nc.sync.dma_start(wo_sb[:], woT)
        cos_sb = consts.tile([P, s], F32R, tag="cos")
        nc.sync.dma_start(cos_sb[:], costab)
        sin_sb = consts.tile([P, s], F32R, tag="sin")
        nc.sync.dma_start(sin_sb[:], sintab)
        pm_sb = consts.tile([P, P], F32R, tag="pm")
        nc.sync.dma_start(pm_sb[:], pmswap)
        ca_sb = consts.tile([P, P], F32R, tag="causal")
        nc.sync.dma_start(ca_sb[:], causal)
        id_sb = consts.tile([P, DK], F32R, tag="ident")
        nc.sync.dma_start(id_sb[:], ident)
        ones_sb = consts.tile([P, 1], F32R, tag="ones")
        nc.sync.dma_start(ones_sb[:], onescol)

        for bi in [i for _ in range(reps) for i in range(b)]:
            # ---- load xT chunks for this batch ----
            xt = []
            for kc in range(NKC):
                t = xpool.tile([P, s], F32R, tag=f"xt{kc}")
                nc.sync.dma_start(t[:], xT[bi, kc])
                xt.append(t)

            # ---- QKV projections -> [128 dims, s] each ----
            qkv_sb = {}
            for name in ("wq", "wk", "wv"):
                dst = qkv.tile([P, s], F32R, tag=f"{name}_sb")
                for nt in range(s // SQT):
                    ps = psum.tile([P, SQT], F32, tag="mm_ps")
                    for kc in range(NKC):
                        nc.tensor.matmul(
                            ps[:],
                            w_sb[name][:, kc],
                            xt[kc][:, nt * SQT : (nt + 1) * SQT],
                            start=(kc == 0),
                            stop=(kc == NKC - 1),
                        )
                    nc.any.tensor_copy(dst[:, nt * SQT : (nt + 1) * SQT], ps[:])
                qkv_sb[name] = dst

            # ---- RoPE on Q^T and K^T (in place) ----
            for name in ("wq", "wk"):
                t_ = qkv_sb[name]
                for nt in range(s // SQT):
                    w = slice(nt * SQT, (nt + 1) * SQT)
                    ps_sw = psum.tile([P, SQT], F32, tag="mm_ps")
                    nc.tensor.matmul(ps_sw[:], pm_sb[:], t_[:, w], start=True, stop=True)
                    t_sin = work.tile([P, SQT], F32R, tag="t_sin")
                    nc.vector.tensor_tensor(
                        t_sin[:], ps_sw[:], sin_sb[:, w], mybir.AluOpType.mult
                    )
                    t_cos = work.tile([P, SQT], F32R, tag="t_cos")
                    nc.vector.tensor_tensor(
                        t_cos[:], t_[:, w], cos_sb[:, w], mybir.AluOpType.mult
                    )
                    nc.vector.tensor_tensor(
                        t_[:, w], t_cos[:], t_sin[:], mybir.AluOpType.add
                    )

            # ---- V: transpose V^T -> V_aug chunks [128 sk, 65] (ones col) ----
            v_aug = []
            for skc in range(n_skc):
                t = qkv.tile([P, HPC, DK + 1], F32R, tag=f"vaug{skc % 16}")
                for h in range(HPC):
                    ps_t = psum.tile([P, DK], F32R, tag="mm_ps")
                    nc.tensor.transpose(
                        ps_t[:],
                        qkv_sb["wv"][h * DK : (h + 1) * DK, skc * P : (skc + 1) * P],
                        id_sb[h * DK : (h + 1) * DK, 0:DK],
                    )
                    nc.any.tensor_copy(t[:, h, 0:DK], ps_t[:])
                    nc.vector.tensor_copy(t[:, h, DK : DK + 1], ones_sb[:])
                v_aug.append(t)

            # ---- attention per sq tile ----
            oT = qkv.tile([P, s], F32R, tag="oT")
            for sqt in range(n_sqt):
                sq0 = sqt * SQT
                nsk = (sq0 + SQT) // P  # causal: sk chunks touching this tile
                po = [
                    opsum.tile([DK + 1, SQT], F32, tag=f"po{h}", name=f"po{h}")
                    for h in range(HPC)
                ]
                for skc in range(nsk):
                    off = max(0, skc * P - sq0)
                    wlen = SQT - off
                    exp_t = [None, None]
                    for h in range(HPC):
                        hd = slice(h * DK, (h + 1) * DK)
                        ps_s = psum.tile([P, SQT], F32, tag=f"score{h}")
                        nc.tensor.matmul(
                            ps_s[:, off:SQT],
                            qkv_sb["wk"][hd, skc * P : (skc + 1) * P],
                            qkv_sb["wq"][hd, sq0 + off : sq0 + SQT],
                            start=True,
                            stop=True,
                        )
                        et = work.tile([P, SQT], F32R, tag=f"exp{h}")
                        nc.scalar.activation(
                            et[:, off:SQT],
                            ps_s[:, off:SQT],
                            mybir.ActivationFunctionType.Exp,
                            scale=float(1.0 / np.sqrt(DK)),
                        )
                        if skc * P >= sq0:  # diagonal chunk: mask the triangle
                            nc.vector.tensor_tensor(
                                et[:, off : off + P],
                                et[:, off : off + P],
                                ca_sb[:],
                                mybir.AluOpType.mult,
                            )
                        exp_t[h] = et
                    for h in range(HPC):
                        nc.tensor.matmul(
                            po[h][:, off:SQT],
                            v_aug[skc][:, h],
                            exp_t[h][:, off:SQT],
                            start=(skc == 0),
                            stop=(skc == nsk - 1),
                        )
                # normalize: oT[h, :, sq] = po[h][0:64, sq] / po[h][64, sq]
                for h in range(HPC):
                    den = work.tile([1, SQT], F32, tag="den")
                    nc.any.tensor_copy(den[:], po[h][DK : DK + 1, :])
                    rec = work.tile([1, SQT], F32, tag="rec")
                    nc.vector.reciprocal(rec[:], den[:])
                    rec_bc = work.tile([DK, SQT], F32, tag="rec_bc")
                    nc.gpsimd.partition_broadcast(rec_bc[:], rec[:])
                    nc.vector.tensor_tensor(
                        oT[h * DK : (h + 1) * DK, sq0 : sq0 + SQT],
                        po[h][0:DK, :],
                        rec_bc[:],
                        mybir.AluOpType.mult,
                    )

            # ---- output projection: out[sq, :] += oT[:, sq].T @ woT ----
            for st in range(n_st):
                ob = work.tile([P, D], F32, tag="ob")
                for nt in range(D // SQT):
                    ps_p = psum.tile([P, SQT], F32, tag="mm_ps")
                    nc.tensor.matmul(
                        ps_p[:],
                        oT[:, st * P : (st + 1) * P],
                        wo_sb[:, nt * SQT : (nt + 1) * SQT],
                        start=True,
                        stop=True,
                    )
                    nc.any.tensor_copy(ob[:, nt * SQT : (nt + 1) * SQT], ps_p[:])
                nc.sync.dma_start(out[bi, st * P : (st + 1) * P, :], ob[:])

    nc.compile()
    return nc


# ---------------- host side ----------------

_ROPE_PERM = None


def _rope_perm():
    """Per-head de-interleave: even dims first, then odd dims."""
    global _ROPE_PERM
    if _ROPE_PERM is None:
        p = []
        for h in range(HPC):
            base = h * DK
            p += [base + 2 * k for k in range(DK // 2)]
            p += [base + 2 * k + 1 for k in range(DK // 2)]
        _ROPE_PERM = np.array(p)
    return _ROPE_PERM


def _host_tables(token_positions, s):
    pos = np.asarray(token_positions).astype(np.float64)
    freqs = THETA ** (-np.arange(0, DK, 2, dtype=np.float64) / DK)  # [32]
    ang = pos[None, :] * freqs[:, None]  # [32, s]
    cos32 = np.cos(ang)
    sin32 = np.sin(ang)
    # layout [128, s]: per head block of 64: [cos32 (x1 half); cos32 (x2 half)]
    cos_t = np.empty((P, s), np.float32)
    sin_t = np.empty((P, s), np.float32)
    for h in range(HPC):
        b0 = h * DK
        cos_t[b0 : b0 + 32] = cos32
        cos_t[b0 + 32 : b0 + 64] = cos32
        sin_t[b0 : b0 + 32] = -sin32  # x1 half: -sin * x2
        sin_t[b0 + 32 : b0 + 64] = sin32  # x2 half: +sin * x1
    # swap permutation matrix (symmetric): swap(j) = j+-32 within each 64-block
    pm = np.zeros((P, P), np.float32)
    for h in range(HPC):
        b0 = h * DK
        for k in range(32):
            pm[b0 + k + 32, b0 + k] = 1.0
            pm[b0 + k, b0 + k + 32] = 1.0
    return cos_t, sin_t, pm


_NC_CACHE = {}

# test harness hooks (off by default; harness calls kernel() directly)
TRACE = False
LAST = {}


def _get_program(b, s, reps=1):
    key = (b, s, reps)
    if key not in _NC_CACHE:
        _NC_CACHE[key] = build_program(b, s, reps)
    return _NC_CACHE[key]


def prepare_in_maps(x, Wq, Wk, Wv, Wo, token_positions):
    x = np.asarray(x, dtype=np.float32)
    Wq = np.asarray(Wq, dtype=np.float32)
    Wk = np.asarray(Wk, dtype=np.float32)
    Wv = np.asarray(Wv, dtype=np.float32)
    Wo = np.asarray(Wo, dtype=np.float32)
    b, s, _ = x.shape

    # [b, kc, p, s] transposed view of x
    xT = np.ascontiguousarray(
        x.transpose(0, 2, 1).reshape(b, NKC, P, s), dtype=np.float32
    )
    cos_t, sin_t, pm = _host_tables(token_positions, s)
    causal = np.triu(np.ones((P, P), np.float32))  # keep p <= f
    ident = np.tile(np.eye(DK, dtype=np.float32), (HPC, 1))

    perm = _rope_perm()
    in_maps = []
    for c in range(NCORES):
        rows = slice(c * BLK, (c + 1) * BLK)
        wq_c = Wq[rows][perm]  # [128, D] rope-permuted rows
        wk_c = Wk[rows][perm]
        wv_c = Wv[rows]
        in_maps.append(
            {
                "xT": xT,
                "wqT": np.ascontiguousarray(wq_c.T.reshape(NKC, P, BLK)),
                "wkT": np.ascontiguousarray(wk_c.T.reshape(NKC, P, BLK)),
                "wvT": np.ascontiguousarray(wv_c.T.reshape(NKC, P, BLK)),
                "woT": np.ascontiguousarray(Wo[:, rows].T),
                "costab": cos_t,
                "sintab": sin_t,
                "pmswap": pm,
                "causal": causal,
                "ident": ident,
                "onescol": np.ones((P, 1), np.float32),
            }
        )

    return in_maps


def kernel(x, Wq, Wk, Wv, Wo, token_positions):
    b, s, _ = np.asarray(x).shape
    nc = _get_program(b, s)
    in_maps = prepare_in_maps(x, Wq, Wk, Wv, Wo, token_positions)
    res = run_bass_kernel_spmd(
        nc, in_maps, core_ids=list(range(NCORES)), trace=TRACE
    )
    LAST["exec_time_ns"] = res.exec_time_ns
    LAST["profile_json"] = res.profile_json
    acc = res.results[0]["out"].astype(np.float32)
    for c in range(1, NCORES):
        acc += res.results[c]["out"]
    return acc
